# revision 64
# baseline (speedup 1.0000x reference)
"""Bidirectional Mamba (PartContextMamba) Trainium2 Bass kernel.

Sharding: pure data parallelism over batch (1024 -> 8 cores x 128 batch).

Token order is (l, b) — time-major — so the L=6 time dim is block-contiguous
(128 tokens per timestep). Per core (feature-major, 768 tokens on free dim):

  xT [768d, 768tok] fp16 (PE transpose of the x shard)
  per direction (fwd, bwd; bwd realized via time-reversed l-block APs):
    xi = W_in_xi @ xT                    (PE fp16, f32 accum)
    xc = Silu(causal dwconv + b)         (DVE taps fp16, one ACT Silu)
    x_dbl = W_xp @ xc -> dt_lo[48] fp16, B[16], C[16] fp16 (single M=80 GEMM)
    B/C staged to DRAM as (t, n, b) then partition-broadcast to brep/crep
    per d-tile mt (full b=128, no half split):
      dt = Softplus(W_dt @ dt_lo + dt_b)   (one ACT op, f32, (t,b) layout)
      pow[t,n,b] = exp(-(n+1) dt[t,b])     (16 ACT exps, t=1..5 only)
      wb[t,n,b]  = (dt*xc) (x) B_rep       (one DVE TT fp16 2x)
      scan unrolled over t: for t=1..5:
        pow_t  = pow_t * h_{t-1}  (TT fp16 2x, in place)
        wb_t  += pow_t            (TT fp16 2x, in place)  -> wb holds h
      Cmult: wb *= crep           (TT fp16 2x, in place)
      n-tree: 4 in-place TT adds halving n -> y at n=0 slice
      ygated = y*? : STT xc*D + y (token order; bwd un-reverses l-blocks)
      z-gate: sz = Silu(W_z @ xT) (ACT), ygated *= sz (TT fp16 2x)
    yout += W_out @ ygated               (PE, PSUM k-accumulation)
  out = LayerNorm(x + yout^T)            (PE transpose, token-major)
"""

import numpy as np

_CACHE: dict = {}

B = 128          # batch per core
L = 6
D = 768
DI = 1536
NT = 12          # d-tiles
NS = 16          # ssm states
R = 48           # dt rank
TOK = B * L
ET = 6           # token-tiles (each = one timestep, 128 tokens)
KT = 6           # k-tiles of D
SV = L * NS * B  # 12288 scan-space free size per d-lane
TNB = NS * B     # 2048 per timestep


GP_CONV = False   # GPSIMD rejects TensorScalarPtr (STT) ops
GP_ZGATE = False  # GPSIMD TT too slow (~6x DVE) + queue stalls
GP_YG = False     # GPSIMD rejects TensorScalarPtr (STT) ops


def _build_module(debug=False):
    import concourse.bass as bass
    import concourse.bacc as bacc
    import concourse.mybir as mybir
    import concourse.tile as tile
    from concourse.masks import make_identity

    f32 = mybir.dt.float32
    f16 = mybir.dt.float16
    AP = bass.AP
    AF = mybir.ActivationFunctionType
    OP = mybir.AluOpType

    nc = bacc.Bacc("TRN2", target_bir_lowering=False)

    x_d = nc.dram_tensor("x", [TOK, D], f32, kind="ExternalInput")
    ins = {}
    for d in ("f", "b"):
        ins[f"win_{d}"] = nc.dram_tensor(f"win_{d}", [D, DI], f16, kind="ExternalInput")
        ins[f"wz_{d}"] = nc.dram_tensor(f"wz_{d}", [NT, 128, KT, 128], f16, kind="ExternalInput")
        ins[f"wxp_{d}"] = nc.dram_tensor(f"wxp_{d}", [128, NT, 96], f16, kind="ExternalInput")
        ins[f"wdt_{d}"] = nc.dram_tensor(f"wdt_{d}", [R, DI], f16, kind="ExternalInput")
        ins[f"wout_{d}"] = nc.dram_tensor(f"wout_{d}", [DI, D], f16, kind="ExternalInput")
        ins[f"aux_{d}"] = nc.dram_tensor(f"aux_{d}", [DI, 8], f32, kind="ExternalInput")
    lng_d = nc.dram_tensor("ln_g", [D], f16, kind="ExternalInput")
    lnb_d = nc.dram_tensor("ln_b", [D], f16, kind="ExternalInput")
    out_d = nc.dram_tensor("out", [TOK, D], f32, kind="ExternalOutput")

    def dram_ap(t, offset, ap):
        return AP(tensor=t, offset=offset, ap=ap)

    def dbg(name, ap):
        if not debug:
            return
        p = ap.partition_size()
        counts = [c for _, c in ap.ap[1:]]
        t = nc.dram_tensor(f"dbg_{name}", [p] + counts, ap.dtype,
                           kind="ExternalOutput")
        nc.sync.dma_start(t[:], ap)

    with tile.TileContext(nc) as tc:
        with (
            tc.tile_pool(name="consts", bufs=1) as consts,
            tc.tile_pool(name="persist", bufs=1) as persist,
            tc.tile_pool(name="wpool", bufs=1) as wpool,
            tc.tile_pool(name="wstream", bufs=2) as wstream,
            tc.tile_pool(name="tr2", bufs=2) as tr2,
            tc.tile_pool(name="tr1", bufs=1) as tr1,
            tc.tile_pool(name="scanp", bufs=1) as scanp,
            tc.tile_pool(name="reps", bufs=1) as repsp,
            tc.tile_pool(name="dram", bufs=1, space="DRAM") as dramp,
            tc.tile_pool(name="psA", bufs=2, space="PSUM") as psA,
            tc.tile_pool(name="psT", bufs=1, space="PSUM") as psT,
            tc.tile_pool(name="psO", bufs=1, space="PSUM") as psO,
        ):
            # ---------------- constants ----------------
            ident = consts.tile([128, 128], f32)
            make_identity(nc, ident)
            identh = consts.tile([128, 128], f16)
            nc.vector.tensor_copy(identh[:], ident[:])
            g_rep = consts.tile([128, D], f16)
            nc.sync.dma_start(g_rep[:], dram_ap(lng_d, 0, [[0, 128], [1, D]]))
            b_rep = consts.tile([128, D], f16)
            nc.sync.dma_start(b_rep[:], dram_ap(lnb_d, 0, [[0, 128], [1, D]]))
            eps_t = consts.tile([128, 1], f32)
            nc.vector.memset(eps_t[:], 1e-5)
            aux = {}
            for d in ("f", "b"):
                aux[d] = consts.tile([128, NT, 8], f32, tag=f"aux_{d}", name=f"aux_{d}")
                nc.sync.dma_start(
                    aux[d][:],
                    dram_ap(ins[f"aux_{d}"], 0, [[8, 128], [8 * 128, NT], [1, 8]]),
                )

            # ---------------- xT (fp16) via PE transpose ----------------
            xT = persist.tile([128, KT, TOK], f16, tag="xT")
            for tt in range(ET):
                xtok = tr1.tile([128, D], f32, tag="xtok")
                nc.sync.dma_start(xtok[:], x_d[tt * 128:(tt + 1) * 128, :])
                for ec in range(KT):
                    pst = psT.tile([128, 128], f32, tag="pst")
                    nc.tensor.transpose(pst[:], xtok[:, ec * 128:(ec + 1) * 128], ident[:])
                    nc.scalar.copy(xT[:, ec, tt * 128:(tt + 1) * 128], pst[:])

            dbg("xT", xT[:])
            yout = persist.tile([128, ET, TOK], f16, tag="yout")
            xc = persist.tile([128, NT, TOK], f16, tag="xc")
            # ygated double-buffered per direction so out_proj(f) can be
            # drip-fed into direction b's scan phase (PE slack there)
            yg = {d: persist.tile([128, NT, TOK], f16, tag=f"ygated_{d}",
                                  name=f"ygated_{d}")
                  for d in ("f", "b")}

            def emit_p4blk(d, dir_i, ng, mg):
                # one out_proj block: yout[:, mg-rows, ng-cols] (+)= W.T @ yg
                ygated = yg[d]
                pso = [psO.tile([128, 384], f32, tag=f"psO{m}", name=f"psO{m}")
                       for m in range(3)]
                for kt in range(NT):
                    wo_t = wstream.tile([128, 3, 128], f16, tag="wo_t")
                    nc.sync.dma_start(
                        wo_t[:],
                        dram_ap(
                            ins[f"wout_{d}"],
                            kt * 128 * D + mg * 384,
                            [[D, 128], [128, 3], [1, 128]],
                        ),
                    )
                    for m in range(3):
                        nc.tensor.matmul(
                            pso[m][:], wo_t[:, m, :],
                            ygated[:, kt, ng * 384:(ng + 1) * 384],
                            start=(kt == 0), stop=(kt == NT - 1),
                        )
                for m in range(3):
                    mt_e = mg * 3 + m
                    o = yout[:, mt_e, ng * 384:(ng + 1) * 384]
                    if dir_i == 0:
                        nc.scalar.copy(o, pso[m][:])
                    else:
                        nc.vector.tensor_tensor(
                            out=o, in0=o, in1=pso[m][:], op=OP.add
                        )

            def emit_phase5(tts):
                # residual + LayerNorm for token tiles tts
                for tt in tts:
                    xtok = tr1.tile([128, D], f32, tag="xtok")
                    nc.sync.dma_start(xtok[:], x_d[tt * 128:(tt + 1) * 128, :])
                    r_t = tr1.tile([128, D], f32, tag="r_t")
                    for ec in range(KT):
                        psh = psT.tile([128, 128], f16, tag="psth")
                        nc.tensor.transpose(
                            psh[:], yout[:, ec, tt * 128:(tt + 1) * 128], identh[:]
                        )
                        nc.vector.tensor_tensor(
                            out=r_t[:, ec * 128:(ec + 1) * 128],
                            in0=psh[:], in1=xtok[:, ec * 128:(ec + 1) * 128],
                            op=OP.add,
                        )
                    stats = tr1.tile([128, 3, nc.vector.BN_STATS_DIM], f32,
                                     tag="stats")
                    for sub in range(3):
                        nc.vector.bn_stats(
                            out=stats[:, sub, :],
                            in_=r_t[:, sub * 256:(sub + 1) * 256]
                        )
                    mv = tr1.tile([128, nc.vector.BN_AGGR_DIM], f32, tag="mv")
                    nc.vector.bn_aggr(out=mv[:], in_=stats[:])
                    rstd = tr1.tile([128, 1], f32, tag="rstd")
                    nc.scalar.activation(
                        out=rstd[:], in_=mv[:, 1:2], func=AF.Sqrt, bias=eps_t[:],
                    )
                    nc.vector.reciprocal(out=rstd[:], in_=rstd[:])
                    nc.vector.tensor_scalar(
                        out=r_t[:], in0=r_t[:], scalar1=mv[:, 0:1],
                        scalar2=rstd[:], op0=OP.subtract, op1=OP.mult,
                    )
                    nc.vector.tensor_tensor(out=r_t[:], in0=r_t[:], in1=g_rep[:],
                                            op=OP.mult)
                    nc.vector.tensor_tensor(out=r_t[:], in0=r_t[:], in1=b_rep[:],
                                            op=OP.add)
                    nc.sync.dma_start(out_d[tt * 128:(tt + 1) * 128, :], r_t[:])

            for dir_i, d in enumerate(("f", "b")):
                fwd = d == "f"

                # x_proj accumulates in dedicated PSUM banks interleaved
                # into phase 1 (one k-slice as each xc tile lands), so its
                # result is ready the moment phase 1 ends
                wxp = wpool.tile([128, NT, 96], f16, tag="wxp")
                nc.sync.dma_start(wxp[:], ins[f"wxp_{d}"][:])
                ps_xp = [psO.tile([128, 384], f32, tag=f"psO{n_}",
                                  name=f"ps_xp{n_}") for n_ in range(2)]
                # ---------------- phase 1: in_proj + conv -> xc ------------
                for mt in range(NT):
                    win = wstream.tile([128, KT, 128], f16, tag="wk")
                    nc.sync.dma_start(
                        win[:],
                        dram_ap(ins[f"win_{d}"], mt * 128,
                                [[DI, 128], [128 * DI, KT], [1, 128]]),
                    )
                    xi_t = tr2.tile([128, L, B], f16, tag="xi")
                    xi_f = xi_t[:].rearrange("p l b -> p (l b)")
                    for ng in range(2):
                        ps = psA.tile([128, 384], f32, tag="psA")
                        for kt in range(KT):
                            nc.tensor.matmul(
                                ps[:],
                                win[:, kt, :],
                                xT[:, kt, ng * 384:(ng + 1) * 384],
                                start=(kt == 0),
                                stop=(kt == KT - 1),
                            )
                        nc.scalar.copy(xi_f[:, ng * 384:(ng + 1) * 384], ps[:])

                    # causal dwconv along l (l-blocks of 128); bwd: reversed l
                    acc = tr2.tile([128, L, B], f16, tag="acc16")
                    cw = [aux[d][:, mt, k:k + 1] for k in range(4)]
                    xi_v = xi_t[:]

                    def xi_rev(cnt):
                        # first `cnt` l-blocks of the time-reversed sequence
                        return AP(
                            tensor=xi_v.tensor, offset=xi_v.offset + 5 * B,
                            ap=[xi_v.ap[0], [-B, cnt], [1, B]],
                        )

                    nc.vector.tensor_scalar(
                        out=acc[:],
                        in0=xi_v if fwd else xi_rev(L),
                        scalar1=cw[3], scalar2=None, op0=OP.mult,
                    )
                    for k in range(3):
                        cnt = k + 3
                        o = acc[:, 3 - k:6, :]
                        nc.vector.scalar_tensor_tensor(
                            out=o,
                            in0=xi_t[:, 0:cnt, :] if fwd else xi_rev(cnt),
                            scalar=cw[k],
                            in1=o, op0=OP.mult, op1=OP.add,
                        )
                    nc.scalar.activation(
                        out=xc[:, mt, :],
                        in_=acc[:].rearrange("p l b -> p (l b)"),
                        func=AF.Silu,
                        bias=aux[d][:, mt, 4:5],
                    )
                    for ng in range(2):
                        nc.tensor.matmul(
                            ps_xp[ng][:96, :],
                            wxp[:, mt, :],
                            xc[:, mt, ng * 384:(ng + 1) * 384],
                            start=(mt == 0),
                            stop=(mt == NT - 1),
                            skip_group_check=True,
                        )

                dbg(f"xc_{d}", xc[:])
                # ---------------- phase 2: x_proj copies + staging ---------
                dt_lo = tr1.tile([R, TOK], f16, tag="dt_lo")
                bc_sb = tr1.tile([32, TOK], f16, tag="bc_sb")
                bstage = dramp.tile([SV], f16, tag="bstage")
                cstage = dramp.tile([SV], f16, tag="cstage")
                for ng in range(2):
                    ps = ps_xp[ng]
                    nc.scalar.copy(dt_lo[:, ng * 384:(ng + 1) * 384], ps[:R, :])
                    nc.scalar.copy(bc_sb[:, ng * 384:(ng + 1) * 384], ps[64:96, :])
                    # stage B/C (SBUF -> DRAM) as (t, n, b) per ng so the
                    # broadcast reload can start before the second ng finishes
                    for part, stg in ((0, bstage), (1, cstage)):
                        sv = stg[:]
                        nc.sync.dma_start(
                            AP(tensor=sv.tensor,
                               offset=sv.offset + ng * 3 * TNB,
                               ap=[[B, 16], [TNB, 3], [1, B]]),
                            bc_sb[part * 16:(part + 1) * 16,
                                  ng * 384:(ng + 1) * 384],
                        )

                dbg(f"dtlo_{d}", dt_lo[:])
                wdt = wpool.tile([R, DI], f16, tag="wdt")
                nc.sync.dma_start(wdt[:], ins[f"wdt_{d}"][:])

                brep = repsp.tile([128, SV], f16, tag="brep")
                nc.sync.dma_start(
                    brep[:],
                    AP(tensor=bstage.tensor, offset=bstage[:].offset,
                       ap=[[0, 128], [1, SV]]),
                )
                crep = repsp.tile([128, SV], f16, tag="crep")
                nc.sync.dma_start(
                    crep[:],
                    AP(tensor=cstage.tensor, offset=cstage[:].offset,
                       ap=[[0, 128], [1, SV]]),
                )

                # ---------------- phase 3: scan (full b, per d-tile) --------
                for mt in range(NT):
                    # dt = softplus(wdt.T @ dt_lo + dt_b): (t,b) = token layout.
                    # fp16 throughout: |pre| <= ~4 on this distribution so
                    # exp(pre) <= ~57 stays far below fp16 max
                    dt_tb = tr2.tile([128, L, B], f16, tag="acc16")
                    dt_f = dt_tb[:].rearrange("p l b -> p (l b)")
                    for ng in range(2):
                        ps = psA.tile([128, 384], f32, tag="psA")
                        nc.tensor.matmul(
                            ps[:], wdt[:, mt * 128:(mt + 1) * 128],
                            dt_lo[:, ng * 384:(ng + 1) * 384],
                            start=True, stop=True,
                        )
                        # softplus = ln(1 + exp(x + dt_b)) via Exp then Ln
                        nc.scalar.activation(
                            out=dt_f[:, ng * 384:(ng + 1) * 384], in_=ps[:],
                            func=AF.Exp, bias=aux[d][:, mt, 5:6],
                        )
                    nc.scalar.activation(
                        out=dt_f, in_=dt_f, func=AF.Ln, bias=1.0,
                    )
                    if mt == 0:
                        dbg(f"dt_{d}", dt_tb[:])

                    # pow[t,n,b] = exp(-(n+1)*dt[t,b]), t=1..5 only (16 ACT exps)
                    powt = scanp.tile([128, L, NS, B], f16, tag="powt")
                    pv = powt[:]
                    for n in range(NS):
                        nc.scalar.activation(
                            out=AP(tensor=pv.tensor,
                                   offset=pv.offset + TNB + n * B,
                                   ap=[pv.ap[0], [TNB, L - 1], [1, B]]),
                            in_=AP(tensor=dt_f.tensor, offset=dt_f.offset + B,
                                   ap=[dt_f.ap[0], [B, L - 1], [1, B]]),
                            func=AF.Exp, scale=-(float(n + 1)),
                        )

                    # wt = dt * xc  (f32 * f16 -> f16)
                    wt_t = tr2.tile([128, L, B], f16, tag="wt_t")
                    nc.vector.tensor_tensor(
                        out=wt_t[:].rearrange("p l b -> p (l b)"),
                        in0=dt_f, in1=xc[:, mt, :], op=OP.mult,
                    )

                    # wb[t,n,b] = wt[t,b] * brep[t,n,b]  (one TT, 2x)
                    wb = scanp.tile([128, L, NS, B], f16, tag="wb")
                    wtv = wt_t[:]
                    wt_bc = AP(
                        tensor=wtv.tensor, offset=wtv.offset,
                        ap=[wtv.ap[0], [B, L], [0, NS], [1, B]],
                    )
                    nc.vector.tensor_tensor(
                        out=wb[:],
                        in0=wt_bc,
                        in1=brep[:].rearrange("p (l n b) -> p l n b", n=NS, b=B),
                        op=OP.mult,
                    )
                    if mt == 0:
                        dbg(f"pow_{d}", powt[:])
                        dbg(f"wb_{d}", wb[:])

                    # unrolled scan over t: wb becomes h
                    wbf = wb[:].rearrange("p l n b -> p (l n b)")
                    pf = powt[:].rearrange("p l n b -> p (l n b)")
                    for t in range(1, L):
                        nc.vector.tensor_tensor(
                            out=pf[:, t * TNB:(t + 1) * TNB],
                            in0=pf[:, t * TNB:(t + 1) * TNB],
                            in1=wbf[:, (t - 1) * TNB:t * TNB],
                            op=OP.mult,
                        )
                        nc.vector.tensor_tensor(
                            out=wbf[:, t * TNB:(t + 1) * TNB],
                            in0=pf[:, t * TNB:(t + 1) * TNB],
                            in1=wbf[:, t * TNB:(t + 1) * TNB],
                            op=OP.add,
                        )
                    if mt == 0:
                        dbg(f"h_{d}", wb[:])

                    # Cmult: g = h * crep (in place in wb; powt is now free,
                    # so the next tile's ACT pow-exps can start immediately)
                    nc.vector.tensor_tensor(
                        out=wbf, in0=wbf, in1=crep[:], op=OP.mult,
                    )
                    # n-tree: 4 in-place halvings in wb; y lands at n=0 slice
                    nh = NS
                    srcf = wbf
                    while nh > 1:
                        nh //= 2
                        a = AP(tensor=wbf.tensor, offset=wbf.offset,
                               ap=[wbf.ap[0], [TNB, L], [1, nh * B]])
                        bb = AP(tensor=wbf.tensor, offset=wbf.offset + nh * B,
                                ap=[wbf.ap[0], [TNB, L], [1, nh * B]])
                        nc.vector.tensor_tensor(out=a, in0=a, in1=bb, op=OP.add)

                    # skip term + token-ordered write into ygated slot
                    og = yg[d][:, mt, :].rearrange("p (l b) -> p l b", b=B)
                    if not fwd:
                        og = AP(tensor=og.tensor, offset=og.offset + 5 * B,
                                ap=[og.ap[0], [-B, L], [1, B]])
                    yfin = AP(tensor=srcf.tensor, offset=srcf.offset,
                              ap=[srcf.ap[0], [TNB, L], [1, B]])
                    # D*xc on ACT (Identity w/ per-partition scale), then a
                    # 2x-mode TT add replaces the 1x-capped STT
                    dxc = tr2.tile([128, L, B], f16, tag="wt_t")
                    nc.scalar.activation(
                        out=dxc[:],
                        in_=xc[:, mt, :].rearrange("p (l b) -> p l b", b=B),
                        func=AF.Identity, scale=aux[d][:, mt, 6:7],
                    )
                    nc.vector.tensor_tensor(
                        out=og, in0=dxc[:], in1=yfin, op=OP.add,
                    )
                    if dir_i == 1 and mt % 3 == 2:
                        # drip one out_proj(f) block into this PE-slack window
                        q = mt // 3
                        emit_p4blk("f", 0, q // 2, q % 2)

                # ---------------- phase 3b: z-silu gate --------------------
                for mt in range(NT):
                    wz_t = wstream.tile([128, KT, 128], f16, tag="wk")
                    nc.sync.dma_start(wz_t[:], ins[f"wz_{d}"][mt, :, :, :])
                    for ng in range(2):
                        ps2 = psA.tile([128, 384], f32, tag="psA")
                        for kt in range(KT):
                            nc.tensor.matmul(
                                ps2[:], wz_t[:, kt, :],
                                xT[:, kt, ng * 384:(ng + 1) * 384],
                                start=(kt == 0), stop=(kt == KT - 1),
                            )
                        sz_t = tr2.tile([128, 384], f16, tag="sz_t")
                        nc.scalar.activation(out=sz_t[:], in_=ps2[:], func=AF.Silu)
                        o = yg[d][:, mt, ng * 384:(ng + 1) * 384]
                        (nc.gpsimd if GP_ZGATE else nc.vector).tensor_tensor(
                            out=o, in0=o, in1=sz_t[:], op=OP.mult,
                        )

                dbg(f"ygated_{d}", yg[d][:])
                # ---------------- phase 4: out_proj ------------------------
                # dir f's blocks were drip-fed into dir b's phase 3 above;
                # dir b's run here with LN interleaved per column group
                if dir_i == 1:
                    for ng in range(2):
                        for mg in range(2):
                            emit_p4blk("b", 1, ng, mg)
                        emit_phase5(range(3 * ng, 3 * ng + 3))

            dbg("yout", yout[:])

    nc.compile()
    return nc


def _prep_inputs(inputs):
    f16 = np.float16
    shared = {}
    for d in ("f", "b"):
        in_proj = np.asarray(inputs[f"{d}_in"], np.float32)      # [3072, 768]
        shared[f"win_{d}"] = np.ascontiguousarray(in_proj[:DI].T).astype(f16)
        wz_T = in_proj[DI:].T                                    # [768, 1536]
        shared[f"wz_{d}"] = np.ascontiguousarray(
            wz_T.reshape(KT, 128, NT, 128).transpose(2, 1, 0, 3)
        ).astype(f16)
        xp_T = np.asarray(inputs[f"{d}_xp"], np.float32).T       # [1536, 80]
        # pad to 96 rows: [dt_lo(48), zeros(16), B(16), C(16)] so PSUM
        # partition reads are 32-aligned
        xp_pad = np.zeros((DI, 96), np.float32)
        xp_pad[:, 0:48] = xp_T[:, 0:48]
        xp_pad[:, 64:96] = xp_T[:, 48:80]
        shared[f"wxp_{d}"] = np.ascontiguousarray(
            xp_pad.reshape(NT, 128, 96).transpose(1, 0, 2)
        ).astype(f16)
        shared[f"wdt_{d}"] = np.ascontiguousarray(
            np.asarray(inputs[f"{d}_dtw"], np.float32).T
        ).astype(f16)                                            # [48, 1536]
        shared[f"wout_{d}"] = np.ascontiguousarray(
            np.asarray(inputs[f"{d}_out"], np.float32).T
        ).astype(f16)                                            # [1536, 768]
        aux = np.zeros((DI, 8), np.float32)
        aux[:, 0:4] = np.asarray(inputs[f"{d}_cw"], np.float32).T
        aux[:, 4] = np.asarray(inputs[f"{d}_cb"], np.float32)
        aux[:, 5] = np.asarray(inputs[f"{d}_dtb"], np.float32)
        aux[:, 6] = np.asarray(inputs[f"{d}_D"], np.float32)
        shared[f"aux_{d}"] = aux
    shared["ln_g"] = np.ascontiguousarray(np.asarray(inputs["ln_g"], np.float16))
    shared["ln_b"] = np.ascontiguousarray(np.asarray(inputs["ln_b"], np.float16))
    return shared


def kernel(**inputs):
    from concourse import bass_utils

    if "nc" not in _CACHE:
        _CACHE["nc"] = _build_module()
    nc = _CACHE["nc"]

    shared = _prep_inputs(inputs)
    x = np.asarray(inputs["x"], np.float32)
    n_cores = 8
    bs = x.shape[0] // n_cores

    in_maps = []
    for c in range(n_cores):
        m = dict(shared)
        # token order (l, b): time-major
        m["x"] = np.ascontiguousarray(
            x[c * bs:(c + 1) * bs].transpose(1, 0, 2).reshape(TOK, D)
        ).astype(np.float32)
        in_maps.append(m)

    res = bass_utils.run_bass_kernel_spmd(nc, in_maps, core_ids=list(range(n_cores)))
    out = np.stack(
        [r["out"].reshape(L, bs, D).transpose(1, 0, 2) for r in res.results], axis=0
    ).reshape(n_cores * bs, L, D)
    return out.astype(np.float32)


# revision 68
# speedup vs baseline: 1.0062x; 1.0062x over previous
"""Bidirectional Mamba (PartContextMamba) Trainium2 Bass kernel.

Sharding: pure data parallelism over batch (1024 -> 8 cores x 128 batch).

Token order is (l, b) — time-major — so the L=6 time dim is block-contiguous
(128 tokens per timestep). Per core (feature-major, 768 tokens on free dim):

  xT [768d, 768tok] fp16 (PE transpose of the x shard)
  per direction (fwd, bwd; bwd realized via time-reversed l-block APs):
    xi = W_in_xi @ xT                    (PE fp16, f32 accum)
    xc = Silu(causal dwconv + b)         (DVE taps fp16, one ACT Silu)
    x_dbl = W_xp @ xc -> dt_lo[48] fp16, B[16], C[16] fp16 (single M=80 GEMM)
    B/C staged to DRAM as (t, n, b) then partition-broadcast to brep/crep
    per d-tile mt (full b=128, no half split):
      dt = Softplus(W_dt @ dt_lo + dt_b)   (one ACT op, f32, (t,b) layout)
      pow[t,n,b] = exp(-(n+1) dt[t,b])     (16 ACT exps, t=1..5 only)
      wb[t,n,b]  = (dt*xc) (x) B_rep       (one DVE TT fp16 2x)
      scan unrolled over t: for t=1..5:
        pow_t  = pow_t * h_{t-1}  (TT fp16 2x, in place)
        wb_t  += pow_t            (TT fp16 2x, in place)  -> wb holds h
      Cmult: wb *= crep           (TT fp16 2x, in place)
      n-tree: 4 in-place TT adds halving n -> y at n=0 slice
      ygated = y*? : STT xc*D + y (token order; bwd un-reverses l-blocks)
      z-gate: sz = Silu(W_z @ xT) (ACT), ygated *= sz (TT fp16 2x)
    yout += W_out @ ygated               (PE, PSUM k-accumulation)
  out = LayerNorm(x + yout^T)            (PE transpose, token-major)
"""

import numpy as np

_CACHE: dict = {}

B = 128          # batch per core
L = 6
D = 768
DI = 1536
NT = 12          # d-tiles
NS = 16          # ssm states
R = 48           # dt rank
TOK = B * L
ET = 6           # token-tiles (each = one timestep, 128 tokens)
KT = 6           # k-tiles of D
SV = L * NS * B  # 12288 scan-space free size per d-lane
TNB = NS * B     # 2048 per timestep


GP_CONV = False   # GPSIMD rejects TensorScalarPtr (STT) ops
GP_ZGATE = False  # GPSIMD TT too slow (~6x DVE) + queue stalls
GP_YG = False     # GPSIMD rejects TensorScalarPtr (STT) ops


def _build_module(debug=False):
    import concourse.bass as bass
    import concourse.bacc as bacc
    import concourse.mybir as mybir
    import concourse.tile as tile
    from concourse.masks import make_identity

    f32 = mybir.dt.float32
    f16 = mybir.dt.float16
    AP = bass.AP
    AF = mybir.ActivationFunctionType
    OP = mybir.AluOpType

    nc = bacc.Bacc("TRN2", target_bir_lowering=False)

    x_d = nc.dram_tensor("x", [TOK, D], f32, kind="ExternalInput")
    ins = {}
    for d in ("f", "b"):
        ins[f"win_{d}"] = nc.dram_tensor(f"win_{d}", [D, DI], f16, kind="ExternalInput")
        ins[f"wz_{d}"] = nc.dram_tensor(f"wz_{d}", [NT, 128, KT, 128], f16, kind="ExternalInput")
        ins[f"wxp_{d}"] = nc.dram_tensor(f"wxp_{d}", [128, NT, 96], f16, kind="ExternalInput")
        ins[f"wdt_{d}"] = nc.dram_tensor(f"wdt_{d}", [R, DI], f16, kind="ExternalInput")
        ins[f"wout_{d}"] = nc.dram_tensor(f"wout_{d}", [DI, D], f16, kind="ExternalInput")
        ins[f"aux_{d}"] = nc.dram_tensor(f"aux_{d}", [DI, 8], f32, kind="ExternalInput")
    lng_d = nc.dram_tensor("ln_g", [D], f16, kind="ExternalInput")
    lnb_d = nc.dram_tensor("ln_b", [D], f16, kind="ExternalInput")
    out_d = nc.dram_tensor("out", [TOK, D], f32, kind="ExternalOutput")

    def dram_ap(t, offset, ap):
        return AP(tensor=t, offset=offset, ap=ap)

    def dbg(name, ap):
        if not debug:
            return
        p = ap.partition_size()
        counts = [c for _, c in ap.ap[1:]]
        t = nc.dram_tensor(f"dbg_{name}", [p] + counts, ap.dtype,
                           kind="ExternalOutput")
        nc.sync.dma_start(t[:], ap)

    with tile.TileContext(nc) as tc:
        with (
            tc.tile_pool(name="consts", bufs=1) as consts,
            tc.tile_pool(name="persist", bufs=1) as persist,
            tc.tile_pool(name="wpool", bufs=1) as wpool,
            tc.tile_pool(name="wstream", bufs=2) as wstream,
            tc.tile_pool(name="tr2", bufs=2) as tr2,
            tc.tile_pool(name="tr1", bufs=1) as tr1,
            tc.tile_pool(name="scanp", bufs=1) as scanp,
            tc.tile_pool(name="reps", bufs=1) as repsp,
            tc.tile_pool(name="dram", bufs=1, space="DRAM") as dramp,
            tc.tile_pool(name="psA", bufs=2, space="PSUM") as psA,
            tc.tile_pool(name="psT", bufs=1, space="PSUM") as psT,
            tc.tile_pool(name="psO", bufs=1, space="PSUM") as psO,
        ):
            # ---------------- constants ----------------
            ident = consts.tile([128, 128], f32)
            make_identity(nc, ident)
            identh = consts.tile([128, 128], f16)
            nc.vector.tensor_copy(identh[:], ident[:])
            g_rep = consts.tile([128, D], f16)
            nc.sync.dma_start(g_rep[:], dram_ap(lng_d, 0, [[0, 128], [1, D]]))
            b_rep = consts.tile([128, D], f16)
            nc.sync.dma_start(b_rep[:], dram_ap(lnb_d, 0, [[0, 128], [1, D]]))
            eps_t = consts.tile([128, 1], f32)
            nc.vector.memset(eps_t[:], 1e-5)
            aux = {}
            for d in ("f", "b"):
                aux[d] = consts.tile([128, NT, 8], f32, tag=f"aux_{d}", name=f"aux_{d}")
                nc.sync.dma_start(
                    aux[d][:],
                    dram_ap(ins[f"aux_{d}"], 0, [[8, 128], [8 * 128, NT], [1, 8]]),
                )

            # ---------------- xT (fp16) via PE transpose ----------------
            xT = persist.tile([128, KT, TOK], f16, tag="xT")
            for tt in range(ET):
                xtok = tr1.tile([128, D], f32, tag="xtok")
                nc.sync.dma_start(xtok[:], x_d[tt * 128:(tt + 1) * 128, :])
                for ec in range(KT):
                    pst = psT.tile([128, 128], f32, tag="pst")
                    nc.tensor.transpose(pst[:], xtok[:, ec * 128:(ec + 1) * 128], ident[:])
                    nc.scalar.copy(xT[:, ec, tt * 128:(tt + 1) * 128], pst[:])

            dbg("xT", xT[:])
            yout = persist.tile([128, ET, TOK], f16, tag="yout")
            xc = persist.tile([128, NT, TOK], f16, tag="xc")
            # ygated double-buffered per direction so out_proj(f) can be
            # drip-fed into direction b's scan phase (PE slack there)
            yg = {d: persist.tile([128, NT, TOK], f16, tag=f"ygated_{d}",
                                  name=f"ygated_{d}")
                  for d in ("f", "b")}

            def emit_p4blk(d, dir_i, ng, mg):
                # one out_proj block: yout[:, mg-rows, ng-cols] (+)= W.T @ yg
                ygated = yg[d]
                pso = [psO.tile([128, 384], f32, tag=f"psO{m}", name=f"psO{m}")
                       for m in range(3)]
                for kt in range(NT):
                    wo_t = wstream.tile([128, 3, 128], f16, tag="wo_t")
                    nc.sync.dma_start(
                        wo_t[:],
                        dram_ap(
                            ins[f"wout_{d}"],
                            kt * 128 * D + mg * 384,
                            [[D, 128], [128, 3], [1, 128]],
                        ),
                    )
                    for m in range(3):
                        nc.tensor.matmul(
                            pso[m][:], wo_t[:, m, :],
                            ygated[:, kt, ng * 384:(ng + 1) * 384],
                            start=(kt == 0), stop=(kt == NT - 1),
                        )
                for m in range(3):
                    mt_e = mg * 3 + m
                    o = yout[:, mt_e, ng * 384:(ng + 1) * 384]
                    if dir_i == 0:
                        nc.scalar.copy(o, pso[m][:])
                    else:
                        nc.vector.tensor_tensor(
                            out=o, in0=o, in1=pso[m][:], op=OP.add
                        )

            def emit_phase5(tts):
                # residual + LayerNorm for token tiles tts
                for tt in tts:
                    xtok = tr1.tile([128, D], f32, tag="xtok")
                    nc.sync.dma_start(xtok[:], x_d[tt * 128:(tt + 1) * 128, :])
                    r_t = tr1.tile([128, D], f32, tag="r_t")
                    for ec in range(KT):
                        psh = psT.tile([128, 128], f16, tag="psth")
                        nc.tensor.transpose(
                            psh[:], yout[:, ec, tt * 128:(tt + 1) * 128], identh[:]
                        )
                        nc.vector.tensor_tensor(
                            out=r_t[:, ec * 128:(ec + 1) * 128],
                            in0=psh[:], in1=xtok[:, ec * 128:(ec + 1) * 128],
                            op=OP.add,
                        )
                    stats = tr1.tile([128, 3, nc.vector.BN_STATS_DIM], f32,
                                     tag="stats")
                    for sub in range(3):
                        nc.vector.bn_stats(
                            out=stats[:, sub, :],
                            in_=r_t[:, sub * 256:(sub + 1) * 256]
                        )
                    mv = tr1.tile([128, nc.vector.BN_AGGR_DIM], f32, tag="mv")
                    nc.vector.bn_aggr(out=mv[:], in_=stats[:])
                    rstd = tr1.tile([128, 1], f32, tag="rstd")
                    nc.scalar.activation(
                        out=rstd[:], in_=mv[:, 1:2], func=AF.Sqrt, bias=eps_t[:],
                    )
                    nc.vector.reciprocal(out=rstd[:], in_=rstd[:])
                    nc.vector.tensor_scalar(
                        out=r_t[:], in0=r_t[:], scalar1=mv[:, 0:1],
                        scalar2=rstd[:], op0=OP.subtract, op1=OP.mult,
                    )
                    nc.vector.tensor_tensor(out=r_t[:], in0=r_t[:], in1=g_rep[:],
                                            op=OP.mult)
                    nc.vector.tensor_tensor(out=r_t[:], in0=r_t[:], in1=b_rep[:],
                                            op=OP.add)
                    nc.sync.dma_start(out_d[tt * 128:(tt + 1) * 128, :], r_t[:])

            for dir_i, d in enumerate(("f", "b")):
                fwd = d == "f"

                # x_proj accumulates in dedicated PSUM banks interleaved
                # into phase 1 (one k-slice as each xc tile lands), so its
                # result is ready the moment phase 1 ends
                wxp = wpool.tile([128, NT, 96], f16, tag="wxp")
                nc.sync.dma_start(wxp[:], ins[f"wxp_{d}"][:])
                ps_xp = [psO.tile([128, 384], f32, tag=f"psO{n_}",
                                  name=f"ps_xp{n_}") for n_ in range(2)]
                # ---------------- phase 1: in_proj + conv -> xc ------------
                for mt in range(NT):
                    win = wstream.tile([128, KT, 128], f16, tag="wk")
                    nc.sync.dma_start(
                        win[:],
                        dram_ap(ins[f"win_{d}"], mt * 128,
                                [[DI, 128], [128 * DI, KT], [1, 128]]),
                    )
                    xi_t = tr2.tile([128, L, B], f16, tag="xi")
                    xi_f = xi_t[:].rearrange("p l b -> p (l b)")
                    for ng in range(2):
                        ps = psA.tile([128, 384], f32, tag="psA")
                        for kt in range(KT):
                            nc.tensor.matmul(
                                ps[:],
                                win[:, kt, :],
                                xT[:, kt, ng * 384:(ng + 1) * 384],
                                start=(kt == 0),
                                stop=(kt == KT - 1),
                            )
                        nc.scalar.copy(xi_f[:, ng * 384:(ng + 1) * 384], ps[:])

                    # causal dwconv along l (l-blocks of 128); bwd: reversed l
                    acc = tr2.tile([128, L, B], f16, tag="acc16")
                    cw = [aux[d][:, mt, k:k + 1] for k in range(4)]
                    xi_v = xi_t[:]

                    def xi_rev(cnt):
                        # first `cnt` l-blocks of the time-reversed sequence
                        return AP(
                            tensor=xi_v.tensor, offset=xi_v.offset + 5 * B,
                            ap=[xi_v.ap[0], [-B, cnt], [1, B]],
                        )

                    # taps 0..2 scaled on ACT into a packed tile (tap k only
                    # needs its first k+3 l-blocks); DVE then does tap 3 + three
                    # 2x-mode adds instead of 1x-capped STTs
                    c012 = tr2.tile([128, 12, B], f16, tag="c012")
                    offs = [0, 3, 7]
                    for k in range(3):
                        cnt = k + 3
                        nc.scalar.activation(
                            out=c012[:, offs[k]:offs[k] + cnt, :],
                            in_=xi_t[:, 0:cnt, :] if fwd else xi_rev(cnt),
                            func=AF.Identity, scale=cw[k],
                        )
                    nc.vector.tensor_scalar(
                        out=acc[:],
                        in0=xi_v if fwd else xi_rev(L),
                        scalar1=cw[3], scalar2=None, op0=OP.mult,
                    )
                    for k in range(3):
                        cnt = k + 3
                        o = acc[:, 3 - k:6, :]
                        nc.vector.tensor_tensor(
                            out=o, in0=o,
                            in1=c012[:, offs[k]:offs[k] + cnt, :], op=OP.add,
                        )
                    nc.scalar.activation(
                        out=xc[:, mt, :],
                        in_=acc[:].rearrange("p l b -> p (l b)"),
                        func=AF.Silu,
                        bias=aux[d][:, mt, 4:5],
                    )
                    for ng in range(2):
                        nc.tensor.matmul(
                            ps_xp[ng][:96, :],
                            wxp[:, mt, :],
                            xc[:, mt, ng * 384:(ng + 1) * 384],
                            start=(mt == 0),
                            stop=(mt == NT - 1),
                            skip_group_check=True,
                        )

                dbg(f"xc_{d}", xc[:])
                # ---------------- phase 2: x_proj copies + staging ---------
                dt_lo = tr1.tile([R, TOK], f16, tag="dt_lo")
                bc_sb = tr1.tile([32, TOK], f16, tag="bc_sb")
                bstage = dramp.tile([SV], f16, tag="bstage")
                cstage = dramp.tile([SV], f16, tag="cstage")
                for ng in range(2):
                    ps = ps_xp[ng]
                    nc.scalar.copy(dt_lo[:, ng * 384:(ng + 1) * 384], ps[:R, :])
                    nc.scalar.copy(bc_sb[:, ng * 384:(ng + 1) * 384], ps[64:96, :])
                    # stage B/C (SBUF -> DRAM) as (t, n, b) per ng so the
                    # broadcast reload can start before the second ng finishes
                    for part, stg in ((0, bstage), (1, cstage)):
                        sv = stg[:]
                        nc.sync.dma_start(
                            AP(tensor=sv.tensor,
                               offset=sv.offset + ng * 3 * TNB,
                               ap=[[B, 16], [TNB, 3], [1, B]]),
                            bc_sb[part * 16:(part + 1) * 16,
                                  ng * 384:(ng + 1) * 384],
                        )

                dbg(f"dtlo_{d}", dt_lo[:])
                wdt = wpool.tile([R, DI], f16, tag="wdt")
                nc.sync.dma_start(wdt[:], ins[f"wdt_{d}"][:])

                brep = repsp.tile([128, SV], f16, tag="brep")
                nc.sync.dma_start(
                    brep[:],
                    AP(tensor=bstage.tensor, offset=bstage[:].offset,
                       ap=[[0, 128], [1, SV]]),
                )
                crep = repsp.tile([128, SV], f16, tag="crep")
                nc.sync.dma_start(
                    crep[:],
                    AP(tensor=cstage.tensor, offset=cstage[:].offset,
                       ap=[[0, 128], [1, SV]]),
                )

                # ---------------- phase 3: scan (full b, per d-tile) --------
                for mt in range(NT):
                    # dt = softplus(wdt.T @ dt_lo + dt_b): (t,b) = token layout.
                    # fp16 throughout: |pre| <= ~4 on this distribution so
                    # exp(pre) <= ~57 stays far below fp16 max
                    dt_tb = tr2.tile([128, L, B], f16, tag="acc16")
                    dt_f = dt_tb[:].rearrange("p l b -> p (l b)")
                    for ng in range(2):
                        ps = psA.tile([128, 384], f32, tag="psA")
                        nc.tensor.matmul(
                            ps[:], wdt[:, mt * 128:(mt + 1) * 128],
                            dt_lo[:, ng * 384:(ng + 1) * 384],
                            start=True, stop=True,
                        )
                        # softplus = ln(1 + exp(x + dt_b)) via Exp then Ln
                        nc.scalar.activation(
                            out=dt_f[:, ng * 384:(ng + 1) * 384], in_=ps[:],
                            func=AF.Exp, bias=aux[d][:, mt, 5:6],
                        )
                    nc.scalar.activation(
                        out=dt_f, in_=dt_f, func=AF.Ln, bias=1.0,
                    )
                    if mt == 0:
                        dbg(f"dt_{d}", dt_tb[:])

                    # pow[t,n,b] = exp(-(n+1)*dt[t,b]), t=1..5 only (16 ACT exps)
                    powt = scanp.tile([128, L, NS, B], f16, tag="powt")
                    pv = powt[:]
                    for n in range(NS):
                        nc.scalar.activation(
                            out=AP(tensor=pv.tensor,
                                   offset=pv.offset + TNB + n * B,
                                   ap=[pv.ap[0], [TNB, L - 1], [1, B]]),
                            in_=AP(tensor=dt_f.tensor, offset=dt_f.offset + B,
                                   ap=[dt_f.ap[0], [B, L - 1], [1, B]]),
                            func=AF.Exp, scale=-(float(n + 1)),
                        )

                    # wt = dt * xc  (f32 * f16 -> f16)
                    wt_t = tr2.tile([128, L, B], f16, tag="wt_t")
                    nc.vector.tensor_tensor(
                        out=wt_t[:].rearrange("p l b -> p (l b)"),
                        in0=dt_f, in1=xc[:, mt, :], op=OP.mult,
                    )

                    # wb[t,n,b] = wt[t,b] * brep[t,n,b]  (one TT, 2x)
                    wb = scanp.tile([128, L, NS, B], f16, tag="wb")
                    wtv = wt_t[:]
                    wt_bc = AP(
                        tensor=wtv.tensor, offset=wtv.offset,
                        ap=[wtv.ap[0], [B, L], [0, NS], [1, B]],
                    )
                    nc.vector.tensor_tensor(
                        out=wb[:],
                        in0=wt_bc,
                        in1=brep[:].rearrange("p (l n b) -> p l n b", n=NS, b=B),
                        op=OP.mult,
                    )
                    if mt == 0:
                        dbg(f"pow_{d}", powt[:])
                        dbg(f"wb_{d}", wb[:])

                    # unrolled scan over t: wb becomes h
                    wbf = wb[:].rearrange("p l n b -> p (l n b)")
                    pf = powt[:].rearrange("p l n b -> p (l n b)")
                    for t in range(1, L):
                        nc.vector.tensor_tensor(
                            out=pf[:, t * TNB:(t + 1) * TNB],
                            in0=pf[:, t * TNB:(t + 1) * TNB],
                            in1=wbf[:, (t - 1) * TNB:t * TNB],
                            op=OP.mult,
                        )
                        nc.vector.tensor_tensor(
                            out=wbf[:, t * TNB:(t + 1) * TNB],
                            in0=pf[:, t * TNB:(t + 1) * TNB],
                            in1=wbf[:, t * TNB:(t + 1) * TNB],
                            op=OP.add,
                        )
                    if mt == 0:
                        dbg(f"h_{d}", wb[:])

                    # Cmult: g = h * crep (in place in wb; powt is now free,
                    # so the next tile's ACT pow-exps can start immediately)
                    nc.vector.tensor_tensor(
                        out=wbf, in0=wbf, in1=crep[:], op=OP.mult,
                    )
                    # n-tree: 4 in-place halvings in wb; y lands at n=0 slice
                    nh = NS
                    srcf = wbf
                    while nh > 1:
                        nh //= 2
                        a = AP(tensor=wbf.tensor, offset=wbf.offset,
                               ap=[wbf.ap[0], [TNB, L], [1, nh * B]])
                        bb = AP(tensor=wbf.tensor, offset=wbf.offset + nh * B,
                                ap=[wbf.ap[0], [TNB, L], [1, nh * B]])
                        nc.vector.tensor_tensor(out=a, in0=a, in1=bb, op=OP.add)

                    # skip term + token-ordered write into ygated slot
                    og = yg[d][:, mt, :].rearrange("p (l b) -> p l b", b=B)
                    if not fwd:
                        og = AP(tensor=og.tensor, offset=og.offset + 5 * B,
                                ap=[og.ap[0], [-B, L], [1, B]])
                    yfin = AP(tensor=srcf.tensor, offset=srcf.offset,
                              ap=[srcf.ap[0], [TNB, L], [1, B]])
                    # D*xc on ACT (Identity w/ per-partition scale), then a
                    # 2x-mode TT add replaces the 1x-capped STT
                    dxc = tr2.tile([128, L, B], f16, tag="wt_t")
                    nc.scalar.activation(
                        out=dxc[:],
                        in_=xc[:, mt, :].rearrange("p (l b) -> p l b", b=B),
                        func=AF.Identity, scale=aux[d][:, mt, 6:7],
                    )
                    nc.vector.tensor_tensor(
                        out=og, in0=dxc[:], in1=yfin, op=OP.add,
                    )
                    if dir_i == 1 and mt % 3 == 2:
                        # drip one out_proj(f) block into this PE-slack window
                        q = mt // 3
                        emit_p4blk("f", 0, q // 2, q % 2)

                # ---------------- phase 3b: z-silu gate --------------------
                for mt in range(NT):
                    wz_t = wstream.tile([128, KT, 128], f16, tag="wk")
                    nc.sync.dma_start(wz_t[:], ins[f"wz_{d}"][mt, :, :, :])
                    for ng in range(2):
                        ps2 = psA.tile([128, 384], f32, tag="psA")
                        for kt in range(KT):
                            nc.tensor.matmul(
                                ps2[:], wz_t[:, kt, :],
                                xT[:, kt, ng * 384:(ng + 1) * 384],
                                start=(kt == 0), stop=(kt == KT - 1),
                            )
                        sz_t = tr1.tile([128, 384], f16, tag="sz_t")
                        nc.scalar.activation(out=sz_t[:], in_=ps2[:], func=AF.Silu)
                        o = yg[d][:, mt, ng * 384:(ng + 1) * 384]
                        (nc.gpsimd if GP_ZGATE else nc.vector).tensor_tensor(
                            out=o, in0=o, in1=sz_t[:], op=OP.mult,
                        )

                dbg(f"ygated_{d}", yg[d][:])
                # ---------------- phase 4: out_proj ------------------------
                # dir f's blocks were drip-fed into dir b's phase 3 above;
                # dir b's run here with LN interleaved per column group
                if dir_i == 1:
                    for ng in range(2):
                        for mg in range(2):
                            emit_p4blk("b", 1, ng, mg)
                        emit_phase5(range(3 * ng, 3 * ng + 3))

            dbg("yout", yout[:])

    nc.compile()
    return nc


def _prep_inputs(inputs):
    f16 = np.float16
    shared = {}
    for d in ("f", "b"):
        in_proj = np.asarray(inputs[f"{d}_in"], np.float32)      # [3072, 768]
        shared[f"win_{d}"] = np.ascontiguousarray(in_proj[:DI].T).astype(f16)
        wz_T = in_proj[DI:].T                                    # [768, 1536]
        shared[f"wz_{d}"] = np.ascontiguousarray(
            wz_T.reshape(KT, 128, NT, 128).transpose(2, 1, 0, 3)
        ).astype(f16)
        xp_T = np.asarray(inputs[f"{d}_xp"], np.float32).T       # [1536, 80]
        # pad to 96 rows: [dt_lo(48), zeros(16), B(16), C(16)] so PSUM
        # partition reads are 32-aligned
        xp_pad = np.zeros((DI, 96), np.float32)
        xp_pad[:, 0:48] = xp_T[:, 0:48]
        xp_pad[:, 64:96] = xp_T[:, 48:80]
        shared[f"wxp_{d}"] = np.ascontiguousarray(
            xp_pad.reshape(NT, 128, 96).transpose(1, 0, 2)
        ).astype(f16)
        shared[f"wdt_{d}"] = np.ascontiguousarray(
            np.asarray(inputs[f"{d}_dtw"], np.float32).T
        ).astype(f16)                                            # [48, 1536]
        shared[f"wout_{d}"] = np.ascontiguousarray(
            np.asarray(inputs[f"{d}_out"], np.float32).T
        ).astype(f16)                                            # [1536, 768]
        aux = np.zeros((DI, 8), np.float32)
        aux[:, 0:4] = np.asarray(inputs[f"{d}_cw"], np.float32).T
        aux[:, 4] = np.asarray(inputs[f"{d}_cb"], np.float32)
        aux[:, 5] = np.asarray(inputs[f"{d}_dtb"], np.float32)
        aux[:, 6] = np.asarray(inputs[f"{d}_D"], np.float32)
        shared[f"aux_{d}"] = aux
    shared["ln_g"] = np.ascontiguousarray(np.asarray(inputs["ln_g"], np.float16))
    shared["ln_b"] = np.ascontiguousarray(np.asarray(inputs["ln_b"], np.float16))
    return shared


def kernel(**inputs):
    from concourse import bass_utils

    if "nc" not in _CACHE:
        _CACHE["nc"] = _build_module()
    nc = _CACHE["nc"]

    shared = _prep_inputs(inputs)
    x = np.asarray(inputs["x"], np.float32)
    n_cores = 8
    bs = x.shape[0] // n_cores

    in_maps = []
    for c in range(n_cores):
        m = dict(shared)
        # token order (l, b): time-major
        m["x"] = np.ascontiguousarray(
            x[c * bs:(c + 1) * bs].transpose(1, 0, 2).reshape(TOK, D)
        ).astype(np.float32)
        in_maps.append(m)

    res = bass_utils.run_bass_kernel_spmd(nc, in_maps, core_ids=list(range(n_cores)))
    out = np.stack(
        [r["out"].reshape(L, bs, D).transpose(1, 0, 2) for r in res.results], axis=0
    ).reshape(n_cores * bs, L, D)
    return out.astype(np.float32)
